# revision 27
# baseline (speedup 1.0000x reference)
"""Trainium2 Bass kernel for nn_Att6 (attention-pooling block).

Computes, for each batch b:
    ht  = tanh(t[b] @ wt)                     (T, H)
    c   = tanh(a[b] @ wa) * tanh(b[b] @ wb) * wh[:, 0]        (H,)
    s   = ht @ c                              (T,)   scores
    att = softmax(s) * mask; att /= sum(att)  (T,)
    out = att @ t[b]                          (D,)

Sharding: data-parallel over batch B=32 across 8 NeuronCores (4 batches
per core), weights replicated. The (T, H) intermediate never leaves the
chip: scores are produced by a second matmul with c as the stationary
operand. Matmuls run as float32r (full-rate fp32 with reduced mantissa).
"""

import sys

sys.path.insert(0, "/opt/trn_rl_repo")

import numpy as np

import bass_rust
import concourse.bass as bass
import concourse.tile as tile
from concourse import mybir
from concourse.masks import make_identity

F32 = mybir.dt.float32
F32R = mybir.dt.float32r
AF = mybir.ActivationFunctionType
AX = mybir.AxisListType

N_CORES = 8
B, T, D, H = 32, 2048, 1024, 1024
BL = B // N_CORES            # batches per core
TCH = 512                    # tau-chunk (columns of one moving matmul)
NTCH = T // TCH              # 4 chunks per batch
NTT = TCH // 128             # 4 tau-tiles per chunk
KD = D // 128                # 8 contraction chunks over D
KH = H // 128                # 8 chunks over H


def split_sync_waits(nc, max_waits=1):
    """This container's walrus accepts only one sem-wait per instruction.
    Move extra waits onto same-engine NOPs inserted immediately before."""
    n_new = 0
    for f in nc.m.functions:
        for bb in f.blocks:
            new = []
            for inst in bb.instructions:
                si = inst.sync_info
                waits = list(si.on_wait) if (si and si.on_wait) else []
                if len(waits) > max_waits:
                    extra, keep = waits[:-max_waits], waits[-max_waits:]
                    for w in extra:
                        nop = bass_rust.InstNoOp(
                            name=f"{inst.name}-sw{n_new}", ins=[], outs=[])
                        nop.engine = inst.engine
                        nop.sync_info = mybir.SyncInfo(on_wait=[w], on_update=[])
                        new.append(nop)
                        n_new += 1
                    si.on_wait = keep
                new.append(inst)
            bb.instructions[:] = new
    return n_new


def build_nc(split_waits=True, reps=1):
    nc = bass.Bass()
    t_in = nc.declare_dram_parameter("t", [BL, T, D], F32, isOutput=False)
    a_in = nc.declare_dram_parameter("a", [BL, D], F32, isOutput=False)
    b_in = nc.declare_dram_parameter("b", [BL, D], F32, isOutput=False)
    m_in = nc.declare_dram_parameter("mask", [BL, T], F32, isOutput=False)
    wt_in = nc.declare_dram_parameter("wt", [D, H], F32, isOutput=False)
    wa_in = nc.declare_dram_parameter("wa", [D, H], F32, isOutput=False)
    wb_in = nc.declare_dram_parameter("wb", [D, H], F32, isOutput=False)
    wh_in = nc.declare_dram_parameter("wh", [H], F32, isOutput=False)
    out_d = nc.declare_dram_parameter("out", [BL, D], F32, isOutput=True)

    with tile.TileContext(nc) as tc:
        _body(nc, tc, t_in, a_in, b_in, m_in, wt_in, wa_in, wb_in, wh_in,
              out_d, reps)
    if split_waits:
        split_sync_waits(nc)
    return nc


def _body(nc, tc, t_in, a_in, b_in, m_in, wt_in, wa_in, wb_in, wh_in, out_d,
          reps):
    with (
        tc.tile_pool(name="const", bufs=1) as const,
        tc.tile_pool(name="wts", bufs=1) as wts,
        tc.tile_pool(name="wab", bufs=4) as wab,
        tc.tile_pool(name="small", bufs=1) as small,
        tc.tile_pool(name="tch", bufs=4) as tch,
        tc.tile_pool(name="tT", bufs=2) as tTp,
        tc.tile_pool(name="hT", bufs=3) as hTp,
        tc.tile_pool(name="rows", bufs=2) as rows,
        tc.tile_pool(name="rowsm", bufs=2) as rowsm,
        tc.tile_pool(name="ps_tr", bufs=2, space="PSUM") as ps_tr,
        tc.tile_pool(name="ps_mm", bufs=2, space="PSUM") as ps_mm,
        tc.tile_pool(name="ps_row", bufs=2, space="PSUM") as ps_row,
    ):
        ident = const.tile([128, 128], F32)
        make_identity(nc, ident)
        identr = const.tile([128, 128], F32R)
        nc.vector.tensor_copy(identr, ident)

        # tiny vector loads + transposes first so PE work exists early
        vT = {}
        for name, vec_in in (("a", a_in), ("b", b_in)):
            v_nat = small.tile([BL, D], F32, tag=f"v{name}")
            nc.sync.dma_start(out=v_nat, in_=vec_in[:, :])
            vT_sb = small.tile([128, KD, BL], F32R, tag=f"vT{name}")
            vT[name] = vT_sb
            for k in range(KD):
                ps = ps_tr.tile([128, BL], F32, tag="tr")
                nc.tensor.transpose(
                    ps, v_nat[:, k * 128:(k + 1) * 128], ident[:BL, :BL])
                nc.vector.tensor_copy(vT_sb[:, k, :], ps)

        # first t-chunk load + transpose before the weight DMAs so the PE
        # and DMA pipelines start immediately
        def emit_chunk_dma(b, j):
            t_nat = tch.tile([128, NTT, D], F32R, tag="tch", name="t_nat")
            nc.sync.dma_start(
                out=t_nat,
                in_=t_in[b, j * TCH:(j + 1) * TCH, :]
                .bitcast(F32R).rearrange("(tt p) d -> p tt d", p=128))
            return t_nat

        def emit_chunk_transposes(t_nat):
            tT_sb = tTp.tile([128, KD, TCH], F32R, tag="tT", name="tT_sb")
            for k in range(KD):
                ps = ps_tr.tile([128, TCH], F32R, tag="tr", name="ps_tr")
                for tt in range(NTT):
                    nc.tensor.transpose(
                        ps[:, tt * 128:(tt + 1) * 128],
                        t_nat[:, tt, k * 128:(k + 1) * 128], identr)
                if k % 2 == 0:
                    nc.vector.tensor_copy(tT_sb[:, k, :], ps)
                else:
                    nc.scalar.copy(tT_sb[:, k, :], ps)
            return tT_sb

        def emit_chunk_load(b, j):
            t_nat = emit_chunk_dma(b, j)
            return t_nat, emit_chunk_transposes(t_nat)

        first_chunk = emit_chunk_load(0, 0)

        # weight DMA order: per h-chunk, wa/wb pair (phase 0 consumes them
        # early) interleaved with the matching wt h-slice (mm1 starts on
        # first slice arrival)
        w_tiles = {}
        predma = {}
        wt_sb = wts.tile([128, KD, H], F32R)
        for hh in range(KH):
            for name, w_in in (("a", wa_in), ("b", wb_in)):
                w_sb = wab.tile(
                    [128, KD, 128], F32R, tag="wsb", name=f"w{name}{hh}")
                nc.sync.dma_start(
                    out=w_sb,
                    in_=w_in[:, hh * 128:(hh + 1) * 128]
                    .bitcast(F32R).rearrange("(k p) h -> p k h", p=128))
                w_tiles[(name, hh)] = w_sb
            nc.sync.dma_start(
                out=wt_sb[:, :, hh * 128:(hh + 1) * 128],
                in_=wt_in[:, hh * 128:(hh + 1) * 128]
                .bitcast(F32R).rearrange("(k p) h -> p k h", p=128))
            if hh == 2:
                # slip batch0-chunk1's t DMA into the weight stream so its
                # data is resident when chunk0's compute finishes
                predma[(0, 1)] = emit_chunk_dma(0, 1)

        whT_sb = const.tile([128, KH], F32)
        nc.sync.dma_start(out=whT_sb, in_=wh_in.rearrange("(k p) -> p k", p=128))

        # ---- phase 0 (h-chunked, interleaved into the first chunk's hh
        # loop): c = tanh(a@wa) * tanh(b@wb) * wh ----
        cT_sb = small.tile([128, KH, BL], F32R)

        def emit_phase0_hh(hh):
            hv = {}
            for name in ("a", "b"):
                w_sb = w_tiles.pop((name, hh))
                ps = ps_row.tile([128, BL], F32, tag="s")
                for k in range(KD):
                    nc.tensor.matmul(
                        ps, w_sb[:, k, :], vT[name][:, k, :],
                        start=(k == 0), stop=(k == KD - 1))
                hv[name] = wab.tile(
                    [128, BL], F32, tag=f"h{name}", name=f"h{name}")
                nc.scalar.activation(hv[name], ps, AF.Tanh)
            prod = wab.tile([128, BL], F32, tag="prod")
            nc.vector.tensor_mul(prod, hv["a"], hv["b"])
            nc.vector.tensor_mul(
                cT_sb[:, hh, :], prod,
                whT_sb[:, hh:hh + 1].to_broadcast([128, BL]))

        # ---- main loop: per chunk, scores -> exp -> mask -> partial
        # pooling accumulate; t chunks release immediately.  No score-max
        # subtraction: |s| <= ||wh||_1 ~ 36 << 88, so exp cannot overflow.
        seq = [(rep, b) for rep in range(reps) for b in range(BL)]
        preloaded = {(0, 0): first_chunk}
        deferred = [None]

        def flush_deferred():
            if deferred[0] is not None:
                fn = deferred[0]
                deferred[0] = None
                fn()

        def make_pool_partial(b, j, t_nat, att_b, ps_out, den_parts, finalize):
            def fn():
                # transpose the 4 e-columns, accumulate the pooling matmul
                attT = rowsm.tile([128, NTT], F32R, tag="attT", name="attT")
                ps_a = ps_tr.tile([128, NTT], F32, tag="tr", name="ps_a")
                for tt in range(NTT):
                    i = j * NTT + tt
                    nc.tensor.transpose(
                        ps_a[:, tt:tt + 1],
                        att_b[:, i * 128:(i + 1) * 128], ident[:1, :1])
                nc.scalar.copy(attT, ps_a)
                for dh in range(2):
                    for tt in range(NTT):
                        nc.tensor.matmul(
                            ps_out[dh], attT[:, tt:tt + 1],
                            t_nat[:, tt, dh * TCH:(dh + 1) * TCH],
                            start=(j == 0 and tt == 0),
                            stop=(j == NTCH - 1 and tt == NTT - 1),
                            skip_group_check=True)
                if finalize:
                    den = rowsm.tile([1, 1], F32, tag="den", name="den")
                    nc.vector.reduce_sum(
                        out=den, in_=den_parts[:, :], axis=AX.X)
                    rden = rowsm.tile([1, 1], F32, tag="rden", name="rden")
                    nc.vector.reciprocal(rden, den)
                    out_b = rows.tile([1, D], F32, tag="orow", name="out_b")
                    for dh in range(2):
                        nc.vector.tensor_scalar_mul(
                            out_b[:, dh * TCH:(dh + 1) * TCH], ps_out[dh], rden)
                    nc.sync.dma_start(out=out_d[b:b + 1, :], in_=out_b)
            return fn

        for idx, (rep, b) in enumerate(seq):
            mask_b = rows.tile([1, T], F32, tag="mrow")
            nc.sync.dma_start(out=mask_b, in_=m_in[b:b + 1, :])
            att_b = rows.tile([1, T], F32, tag="arow")
            den_parts = rowsm.tile([1, NTCH], F32, tag="denp")
            ps_out = [ps_row.tile([1, TCH], F32, tag="o", name=f"o{dh}")
                      for dh in range(2)]
            for j in range(NTCH):
                key = (idx, j)
                if key in preloaded:
                    t_nat, tT_sb = preloaded.pop(key)
                elif key in predma:
                    t_nat = predma.pop(key)
                    tT_sb = emit_chunk_transposes(t_nat)
                else:
                    t_nat, tT_sb = emit_chunk_load(b, j)

                sl = slice(j * TCH, (j + 1) * TCH)
                ps_s = ps_row.tile([1, TCH], F32, tag="s")
                for hh in range(KH):
                    ps_h = ps_mm.tile([128, TCH], F32, tag="mm1")
                    for k in range(KD):
                        nc.tensor.matmul(
                            ps_h, wt_sb[:, k, hh * 128:(hh + 1) * 128],
                            tT_sb[:, k, :], start=(k == 0), stop=(k == KD - 1))
                    hT = hTp.tile([128, TCH], F32R, tag="hT")
                    nc.scalar.activation(hT, ps_h, AF.Tanh)
                    if idx == 0 and j == 0:
                        emit_phase0_hh(hh)
                    nc.tensor.matmul(
                        ps_s, cT_sb[:, hh, b:b + 1], hT,
                        start=(hh == 0), stop=(hh == KH - 1),
                        skip_group_check=True)

                # e = exp(scores) fused with the PSUM->SBUF move
                nc.scalar.activation(att_b[:, sl], ps_s, AF.Exp)
                nc.vector.tensor_mul(att_b[:, sl], att_b[:, sl], mask_b[:, sl])
                nc.vector.reduce_sum(
                    out=den_parts[:, j:j + 1], in_=att_b[:, sl], axis=AX.X)

                flush_deferred()
                deferred[0] = make_pool_partial(
                    b, j, t_nat, att_b, ps_out, den_parts,
                    finalize=(j == NTCH - 1))
        flush_deferred()


_NC = None


def _get_nc():
    global _NC
    if _NC is None:
        _NC = build_nc()
    return _NC


def _shard_inputs(t, a, b, mask, wt, wa, wb, wh):
    t = np.asarray(t, dtype=np.float32)
    a = np.asarray(a, dtype=np.float32)
    b = np.asarray(b, dtype=np.float32)
    mask_f = np.asarray(mask).astype(np.float32)
    wt = np.ascontiguousarray(np.asarray(wt, dtype=np.float32))
    wa = np.ascontiguousarray(np.asarray(wa, dtype=np.float32))
    wb = np.ascontiguousarray(np.asarray(wb, dtype=np.float32))
    wh = np.ascontiguousarray(np.asarray(wh, dtype=np.float32).reshape(H))
    in_maps = []
    for c in range(N_CORES):
        sl = slice(BL * c, BL * (c + 1))
        in_maps.append({
            "t": np.ascontiguousarray(t[sl]),
            "a": np.ascontiguousarray(a[sl]),
            "b": np.ascontiguousarray(b[sl]),
            "mask": np.ascontiguousarray(mask_f[sl]),
            "wt": wt, "wa": wa, "wb": wb, "wh": wh,
        })
    return in_maps


def kernel(t, a, b, mask, wt, wa, wb, wh):
    from concourse.bass_utils import run_bass_kernel_spmd

    nc = _get_nc()
    in_maps = _shard_inputs(t, a, b, mask, wt, wa, wb, wh)
    res = run_bass_kernel_spmd(nc, in_maps, core_ids=list(range(N_CORES)))
    out = np.concatenate([res.results[c]["out"] for c in range(N_CORES)], axis=0)
    return np.ascontiguousarray(out, dtype=np.float32)
